# revision 28
# baseline (speedup 1.0000x reference)
"""TRN2 Bass kernel for nn_MultiHeadSelfAttention_15822659518596.

Softmax and V are dead code in the reference; the output collapses to

    out_b = q_b @ M_b + b_fc,   q_b = x_b @ Wq.T + bq            (S, D)
    M_b[c,o] = sum_j Kf_b[j,c] * Wfc[o, j*8+n(c)] / 8            (D, D)
    Kf_b = x_b @ Wk.T + bk                                       (S, D)

Sharding: 8 cores = (4 batches) x (2 c-halves).  Core (b, h) owns heads
4h..4h+3 (c-half), computes Kf/qT for those c columns, builds the full
M[c-half, :] rows, and emits a PARTIAL output (contraction over its
c-half).  The two partials per batch are summed on the host (O(bytes)).
No device collectives; no duplicated matmul work across cores:
65,536 PE cycles/core (the 8-way-optimal count).

Precision: Wfc is shipped as fp8 e3m4 scaled by 512 (values sit in the
e3m4 normal range); the inverse 1/512 is folded into Wq/bq on the host,
so out = (q/512) @ (512*M) needs no on-chip rescaling.  Everything else
runs bf16 with f32 psum.  Measured end-to-end frobenius rel err ~1.4e-2
(budget 2e-2).
"""

import ml_dtypes
import numpy as np

import concourse.bass as bass
import concourse.tile as tile
from concourse import mybir, bacc
from concourse.bass_utils import run_bass_kernel_spmd

B, S, D, H = 4, 2048, 512, 8
DK = D // H            # 64
CH = D // 2            # 256 c-columns per core (4 heads)
NC = 8
F32 = mybir.dt.float32
BF16 = mybir.dt.bfloat16
FP8E3 = mybir.dt.float8e3
COPY = mybir.ActivationFunctionType.Identity
WFC_SCALE = 512.0

_CACHE = {}


def _build_program():
    """One SPMD Bass program; per-core tensors differ only in data."""
    nc = bacc.Bacc("TRN2", target_bir_lowering=False, debug=False, num_devices=NC)

    # x_b.T packed as [128, (jh 2) x (dt 4) x 1024]: col h*4096 + dt*1024 + c
    # holds x_b[h*1024 + c, dt*128 + p] -- two contiguous 8KB-line DMAs
    xT2 = nc.dram_tensor("xT2", [128, 8192], BF16, kind="ExternalInput")
    # packed: cols [0,1024) = Wk.T d-tile slices, [1024,2048) = (Wq/512).T
    wqk = nc.dram_tensor("wqk", [128, 2048], BF16, kind="ExternalInput")
    # per local head: [j-part 128, jt 16 x o 512], e3m4, scaled by 512/8
    wfc8 = nc.dram_tensor("wfc8", [4, 128, 16 * D], FP8E3, kind="ExternalInput")
    bqt = nc.dram_tensor("bqt", [128, 2], F32, kind="ExternalInput")    # (bq/512) cols
    bkrow = nc.dram_tensor("bkrow", [1, CH], BF16, kind="ExternalInput")
    colsum = nc.dram_tensor("colsum", [1, 4 * D], BF16, kind="ExternalInput")
    outP = nc.dram_tensor("outP", [D, S], BF16, kind="ExternalOutput")  # partial out.T

    with tile.TileContext(nc) as tc:
        with tc.tile_pool(name="xt", bufs=4) as p_xt, \
             tc.tile_pool(name="w", bufs=1) as p_w, \
             tc.tile_pool(name="kf", bufs=16) as p_kf, \
             tc.tile_pool(name="qt", bufs=2) as p_qt, \
             tc.tile_pool(name="m", bufs=2) as p_m, \
             tc.tile_pool(name="wf", bufs=4) as p_wf, \
             tc.tile_pool(name="ob", bufs=3) as p_ob, \
             tc.tile_pool(name="bias", bufs=1) as p_bias, \
             tc.tile_pool(name="ps1", bufs=2, space="PSUM") as ps1, \
             tc.tile_pool(name="ps2", bufs=1, space="PSUM") as ps2, \
             tc.tile_pool(name="ps3", bufs=2, space="PSUM") as ps3, \
             tc.tile_pool(name="ps4", bufs=3, space="PSUM") as ps4:

            # ---- input DMAs, in stream-priority order: weights + first x
            # half (stage 1 starts ASAP; each trigger costs ~0.6us on Sync,
            # so the tiny bias loads come after), then the rest of x, then
            # the big wfc stream whose tail gates the kernel ----
            t_w = p_w.tile([128, 2048], BF16, tag="w")
            nc.sync.dma_start(t_w[:], wqk[:])

            t_x = p_xt.tile([128, 8192], BF16, tag="xt")
            nc.sync.dma_start(t_x[:, :4096], xT2[:, :4096])

            t_bq = p_bias.tile([128, 2], F32, tag="bq")
            nc.sync.dma_start(t_bq[:], bqt[:])
            t_bk = p_bias.tile([1, CH], BF16, tag="bk")
            nc.sync.dma_start(t_bk[:], bkrow[:])
            t_cs = p_bias.tile([1, 4 * D], BF16, tag="cs")
            nc.sync.dma_start(t_cs[:], colsum[:])

            nc.sync.dma_start(t_x[:, 4096:], xT2[:, 4096:])

            wfs = []
            for ln in range(4):
                t_wf = p_wf.tile([128, 16 * D], FP8E3, tag="wf")
                nc.sync.dma_start(t_wf[:], wfc8[ln][:])
                wfs.append(t_wf)

            def xs(di, c, width):
                """x_b.T slice [d-tile di, x-rows c:c+width] in t_x."""
                jh, off = divmod(c, 1024)
                base = jh * 4096 + di * 1024 + off
                return t_x[:, base:base + width]

            # ---- stage 1: Kf[j, c-half] (16 j-tiles) ----
            kfs = []
            for jt in range(16):
                pk = ps1.tile([128, CH], F32)
                for di in range(4):
                    nc.tensor.matmul(
                        pk[:], xs(di, jt * 128, 128),
                        t_w[:, di * 256:(di + 1) * 256],
                        start=(di == 0), stop=(di == 3))
                t_kf = p_kf.tile([128, CH], BF16, tag="kf")
                nc.vector.tensor_copy(t_kf[:], pk[:])
                kfs.append(t_kf)

            # ---- stage 3: qT[c-half, i] scaled by 1/512 ----
            qts = [p_qt.tile([128, S], BF16, tag="qt", name=f"t_q{ct}")
                   for ct in range(2)]
            for ic in range(4):
                for ct in range(2):
                    pq = ps3.tile([128, 512], F32)
                    for di in range(4):
                        nc.tensor.matmul(
                            pq[:],
                            t_w[:, 1024 + di * 256 + ct * 128:
                                 1024 + di * 256 + (ct + 1) * 128],
                            xs(di, ic * 512, 512),
                            start=(di == 0), stop=(di == 3))
                    nc.scalar.activation(
                        qts[ct][:, ic * 512:(ic + 1) * 512], pq[:], COPY,
                        bias=t_bq[:, ct:ct + 1])

            # ---- stage 2: M rows for this c-half, one psum per head pair.
            # bf16/fp8 matmuls support PE column-group tiling: head 2u in
            # psum[0:64] (col group 0), head 2u+1 in psum[64:128] (group 64),
            # concurrent in the array.  memset + start=False everywhere so
            # the two disjoint chains can't WAW-race on bank clears. ----
            ms = []
            for u in range(2):
                n0, n1 = 2 * u, 2 * u + 1
                pm = ps2.tile([128, D], F32)
                nc.vector.memset(pm[:], 0.0)
                # exact b_qkv k-bias first (M += bk[c] (x) colsum_n[o]) so
                # the chain tail -- which gates the M copy -- is a j-tile
                # matmul, not this rank-1 straggler
                nc.tensor.matmul(
                    pm[0:64, :], t_bk[0:1, n0 * 64:(n0 + 1) * 64],
                    t_cs[0:1, n0 * D:(n0 + 1) * D],
                    start=False, stop=False, tile_position=(0, 0),
                    skip_group_check=True)
                nc.tensor.matmul(
                    pm[64:128, :], t_bk[0:1, n1 * 64:(n1 + 1) * 64],
                    t_cs[0:1, n1 * D:(n1 + 1) * D],
                    start=False, stop=False, tile_position=(0, 64),
                    skip_group_check=True)
                for jt in range(16):
                    nc.tensor.matmul(
                        pm[0:64, :], kfs[jt][:, n0 * 64:(n0 + 1) * 64],
                        wfs[n0][:, jt * D:(jt + 1) * D],
                        start=False, stop=(jt == 15), tile_position=(0, 0),
                        skip_group_check=True)
                    nc.tensor.matmul(
                        pm[64:128, :], kfs[jt][:, n1 * 64:(n1 + 1) * 64],
                        wfs[n1][:, jt * D:(jt + 1) * D],
                        start=False, stop=(jt == 15), tile_position=(0, 64),
                        skip_group_check=True)
                t_m = p_m.tile([128, D], BF16, tag="m")
                # two half-casts: the first unblocks stage 4's ot 0/1 sooner
                nc.vector.tensor_copy(t_m[:, :256], pm[:, :256])
                nc.vector.tensor_copy(t_m[:, 256:], pm[:, 256:])
                ms.append(t_m)

            # ---- stage 4: partial outT[o, i] = sum_{c-half} M.T-contract.
            # 4 psum bufs + casts alternating DVE/ACT keep the drain off the
            # matmul critical path ----
            for ot in range(4):
                t_o = p_ob.tile([128, S], BF16, tag="ob")
                for ic in range(4):
                    po = ps4.tile([128, 512], F32)
                    for u in range(2):
                        nc.tensor.matmul(
                            po[:], ms[u][:, ot * 128:(ot + 1) * 128],
                            qts[u][:, ic * 512:(ic + 1) * 512],
                            start=(u == 0), stop=(u == 1))
                    if ic % 2 == 0:
                        nc.vector.tensor_copy(t_o[:, ic * 512:(ic + 1) * 512], po[:])
                    else:
                        nc.scalar.activation(
                            t_o[:, ic * 512:(ic + 1) * 512], po[:], COPY)
                        nc.sync.dma_start(
                            outP[ot * 128:(ot + 1) * 128,
                                 (ic - 1) * 512:(ic + 1) * 512],
                            t_o[:, (ic - 1) * 512:(ic + 1) * 512])
    nc.compile()
    return nc


def _prep_inputs(x, W_qkv, b_qkv, W_fc, b_fc):
    """Host-side sharding/layout prep. O(bytes) only -- no GEMM work."""
    x = np.ascontiguousarray(x, dtype=np.float32)
    W_qkv = np.asarray(W_qkv, dtype=np.float32)
    b_qkv = np.asarray(b_qkv, dtype=np.float32)
    W_fc = np.asarray(W_fc, dtype=np.float32)

    wq3 = W_qkv.reshape(H, 3, DK, D)          # [n, {q,k,v}, kk, d]
    wq = wq3[:, 0].reshape(D, D)              # [c, d], c = n*64+kk
    wk = wq3[:, 1].reshape(D, D)
    bq3 = b_qkv.reshape(H, 3, DK)
    bq_c = np.ascontiguousarray(bq3[:, 0].reshape(D))
    bk_c = np.ascontiguousarray(bq3[:, 1].reshape(D))

    # [j, n, o] view of Wfc scaled by 512/8; e3m4 wants values ~O(1)
    G = np.ascontiguousarray((W_fc * (WFC_SCALE / 8.0)).T).reshape(S, H, D)

    in_maps = [dict() for _ in range(NC)]
    for b in range(B):
        xt = x[b].T                               # [512 d, 2048 j]
        xT2_b = np.ascontiguousarray(
            xt.reshape(4, 128, 2, 1024).transpose(1, 2, 0, 3).reshape(128, 8192)
        ).astype(ml_dtypes.bfloat16)
        in_maps[2 * b]["xT2"] = xT2_b
        in_maps[2 * b + 1]["xT2"] = xT2_b
    for h in range(2):
        cs, ce = h * CH, (h + 1) * CH
        wqT = np.ascontiguousarray((wq[cs:ce, :] / WFC_SCALE).T)  # [d, 256]
        wkT = np.ascontiguousarray(wk[cs:ce, :].T)
        wpack = np.empty((128, 2048), np.float32)
        for dt in range(4):
            wpack[:, dt * 256:(dt + 1) * 256] = wkT[dt * 128:(dt + 1) * 128, :]
            wpack[:, 1024 + dt * 256:1024 + (dt + 1) * 256] = \
                wqT[dt * 128:(dt + 1) * 128, :]
        wpack = wpack.astype(ml_dtypes.bfloat16)
        bqt = np.ascontiguousarray(
            (bq_c[cs:ce] / WFC_SCALE).reshape(2, 128).T).astype(np.float32)
        bkrow = bk_c[cs:ce].reshape(1, CH).astype(ml_dtypes.bfloat16)

        wfc8 = np.empty((4, 128, 16 * D), ml_dtypes.float8_e3m4)
        csum = np.empty((1, 4 * D), np.float32)
        for ln in range(4):
            n = 4 * h + ln
            head = G[:, n, :]                 # [2048 j, 512 o]
            t8 = np.ascontiguousarray(
                head.reshape(16, 128, D).transpose(1, 0, 2).reshape(128, 16 * D)
            ).astype(ml_dtypes.float8_e3m4)
            wfc8[ln] = t8
            # colsum over the QUANTIZED values so the bk rank-1 term is exact
            csum[0, ln * D:(ln + 1) * D] = (
                t8.astype(np.float32).reshape(128, 16, D).sum(axis=(0, 1)))
        csum_b = csum.astype(ml_dtypes.bfloat16)

        for b in range(B):
            in_maps[2 * b + h].update({
                "wqk": wpack, "wfc8": wfc8, "bqt": bqt,
                "bkrow": bkrow, "colsum": csum_b,
            })
    return in_maps, np.asarray(b_fc, dtype=np.float32)


def _run(in_maps, trace=False, **kw):
    if "nc" not in _CACHE:
        _CACHE["nc"] = _build_program()
    return run_bass_kernel_spmd(
        _CACHE["nc"], in_maps, core_ids=list(range(NC)), trace=trace, **kw)


def _assemble(results, b_fc):
    out = np.empty((B, S, D), dtype=np.float32)
    for b in range(B):
        p = (results[2 * b]["outP"].astype(np.float32)
             + results[2 * b + 1]["outP"].astype(np.float32))
        out[b] = p.T + b_fc
    return out


def kernel(x, W_qkv, b_qkv, W_fc, b_fc):
    in_maps, bfc = _prep_inputs(x, W_qkv, b_qkv, W_fc, b_fc)
    res = _run(in_maps, trace=False)
    return _assemble(res.results, bfc)


def kernel_traced(x, W_qkv, b_qkv, W_fc, b_fc):
    """Like kernel() but returns (out, BassKernelResults) with NTFF trace."""
    import os
    os.environ.setdefault("BASS_PERFETTO_PROFILE_ALL_CORES", "1")
    _install_ntff_hook_shim()
    in_maps, bfc = _prep_inputs(x, W_qkv, b_qkv, W_fc, b_fc)
    res = _run(in_maps, trace=True)
    return _assemble(res.results, bfc), res


def _install_ntff_hook_shim():
    """The agent image's antenv lacks axon_hooks; provide it so
    run_bass_kernel_spmd(trace=True) can reach the NTFF profiler."""
    import sys, types
    if "antenv.axon_hooks" in sys.modules:
        return
    try:
        from trn_agent_boot.trn_boot import _ntff_profile_via_ctypes
    except ImportError:
        return
    mod = types.ModuleType("antenv.axon_hooks")
    _hook = [None]
    mod.set_axon_ntff_profile_hook = lambda h: _hook.__setitem__(0, h)
    mod.get_axon_ntff_profile_hook = lambda: _hook[0]
    import antenv
    sys.modules["antenv.axon_hooks"] = mod
    antenv.axon_hooks = mod
    so = "/opt/axon/libaxon_pjrt.so"
    try:
        hook = _ntff_profile_via_ctypes(so)
    except OSError:
        hook = None
    mod.set_axon_ntff_profile_hook(hook)
